# revision 8
# baseline (speedup 1.0000x reference)
"""Trainium2 Bass kernel for a single-head cross-attention block.

Reference computation (per batch b of B=128):
    q = input[b] @ Wq            # [T,H]   T=512, C=384, H=64
    k = x[b] @ Wk                # [T,H]
    v = x[b] @ Wv                # [T,H]
    S = (q @ k.T) * C**-0.5      # [T,T], causal mask
    P = softmax(S, axis=-1)
    out[b] = P @ v               # [T,H]

Data-parallel over 8 NeuronCores (16 batches each).  Key structure:

  * Host pre-packs input+x for a PAIR of batches into one DRAM blob laid
    out so every SBUF partition's slice is a single 12 KiB contiguous run;
    one dma_start per pair (DMA issue costs ~600ns of engine time each,
    and small packets choke the queue dispatcher).  Outputs likewise
    store bf16 pair-blobs.  All DMA issues ride the (otherwise idle)
    sync-engine HWDGE queue.
  * k and v projections are fused into one matmul pass using a stacked
    [Wk|Wv] stationary tile: out [128,T] holds kT on partitions 0-63 and
    vT on 64-127 (halves the kv projection PE cost).  Odd batches use
    [Wv|Wk] so kT lands on partitions 64-127 - this matches the q
    projection of odd batches which is PSUM-stacked on partitions 64-127
    (two batches share one PSUM bank + one PSUM->SBUF copy).
  * v is re-oriented [t,h] with 4 tiny PE transposes (bf16, via identity).
  * S^T chunks exploit causality (chunk m covers queries >= 128m);
    exp runs on ScalarE in 2 instructions (chunk 0, chunks 1+3+2 packed
    into one PSUM tile); diagonal-block masks multiply by a 0/1 upper-
    triangular tile on DVE.
  * P@V accumulates with an extra ones-column on v so the softmax
    denominator falls out of the same matmuls; normalization is a
    broadcast multiply by the reciprocal.
  * GPSIMD cannot touch PSUM (walrus verifier), so PSUM->SBUF moves
    split between DVE and ScalarE.  Software pipelining: q projections
    run two batches ahead, kv one batch ahead, so the PE stream stays
    dense while exp/mask chase it.
"""

import numpy as np
import ml_dtypes

import concourse.bass as bass
import concourse.tile as tile
import concourse.mybir as mybir
from concourse.bass import broadcast_tensor_aps
from concourse.bass_utils import run_bass_kernel_spmd
from concourse.masks import make_upper_triangular, make_identity

N_CORES = 8
B, T, C, H = 128, 512, 384, 64
BPC = B // N_CORES          # batches per core
NP = BPC // 2               # batch pairs per core
CK = C // 128               # contraction chunks for projections
TK = T // 128               # T chunks
SCALE = float(C) ** -0.5
BF16 = mybir.dt.bfloat16
F32 = mybir.dt.float32
EXP = mybir.ActivationFunctionType.Exp
MULT = mybir.AluOpType.mult

_bf16 = ml_dtypes.bfloat16

# weight tile column layout: [Wk|Wv] (even), [Wv|Wk] (odd), Wq
WKV_E, WKV_O, WQ0 = 0, 128, 256
WCOLS = 320


def _split_multi_waits(nc: bass.Bass):
    """walrus in this build encodes at most ONE sync-wait per instruction.
    Tile's wait-assignment can attach several. Move the extras onto
    same-engine NOPs inserted immediately before each instruction —
    identical semantics (the engine blocks on the NOP waits first)."""
    n = 0
    for bb in nc.m.functions[0].blocks:
        new_insts = []
        for inst in bb.instructions:
            si = inst.sync_info
            waits = list(si.on_wait) if si and si.on_wait else []
            if len(waits) > 1:
                for w in waits[:-1]:
                    nop = mybir.InstNoOp(name=f"WSPLIT-{n}", ins=[], outs=[])
                    n += 1
                    nop.engine = inst.engine
                    nop.sync_info = mybir.SyncInfo(on_wait=[w], on_update=[])
                    new_insts.append(nop)
                si.on_wait = waits[-1:]
            new_insts.append(inst)
        bb.instructions[:] = new_insts


def build_kernel(split_waits: bool = True) -> bass.Bass:
    nc = bass.Bass()
    # fused[pair, p, (j, s, c, t)]: batch 2*pair+j, s=0 input / s=1 x,
    # value = src[b, t, c*128 + p]
    fused = nc.dram_tensor("fused", [NP, 128, 2 * 2 * CK * T], BF16,
                           kind="ExternalInput")
    w = nc.dram_tensor("w", [128, CK * WCOLS], BF16, kind="ExternalInput")
    out = nc.dram_tensor("out", [NP, 128, 2 * TK * H], BF16,
                         kind="ExternalOutput")

    with tile.TileContext(nc) as tc:
        with (
            tc.tile_pool(name="const", bufs=1) as const_pool,
            tc.tile_pool(name="ld", bufs=4) as ld_pool,
            tc.tile_pool(name="kv", bufs=2) as kv_pool,
            tc.tile_pool(name="qsb", bufs=2) as q_pool,
            tc.tile_pool(name="vsb", bufs=2) as v_pool,
            tc.tile_pool(name="e", bufs=2) as e_pool,
            tc.tile_pool(name="osb", bufs=2) as o_pool,
            tc.tile_pool(name="rcp", bufs=2) as r_pool,
            tc.tile_pool(name="q_ps", bufs=1, space="PSUM") as q_psum,
            tc.tile_pool(name="kv_ps", bufs=1, space="PSUM") as kv_psum,
            tc.tile_pool(name="vt_ps", bufs=1, space="PSUM") as vt_psum,
            tc.tile_pool(name="st_ps", bufs=1, space="PSUM") as st_psum,
            tc.tile_pool(name="o_ps", bufs=2, space="PSUM") as o_psum,
        ):
            # ---- constants ----
            w_sb = const_pool.tile([128, CK, WCOLS], BF16, tag="w")
            nc.sync.dma_start(w_sb[:], w[:].rearrange("p (c f) -> p c f", c=CK))
            tri = const_pool.tile([128, 128], BF16, tag="tri")
            make_upper_triangular(nc, tri[:], val=1.0, diag=True)
            tripair = const_pool.tile([128, 2, 128], BF16, tag="tripair")
            make_upper_triangular(nc, tripair[:, 0, :], val=1.0, diag=True)
            make_upper_triangular(nc, tripair[:, 1, :], val=1.0, diag=True)
            # identity stacked twice so both parities can transpose from
            # their base partition (0 or 64)
            ident = const_pool.tile([128, 64], BF16, tag="ident")
            make_identity(nc, ident[0:64, :])
            make_identity(nc, ident[64:128, :])

            lds, kvs, qps_t, qsbs, osbs = {}, {}, {}, {}, {}

            def ld_view(i):
                # [s, c, t] view (128 partitions) of batch i in its pair tile
                return lds[i // 2][:, i % 2]

            def emit_ld(pair):
                if pair >= NP or pair in lds:
                    return
                t_ = ld_pool.tile([128, 2, 2, CK, T], BF16, tag="ld", name="ld")
                nc.sync.dma_start(
                    t_[:],
                    fused[pair].rearrange("p (j s c t) -> p j s c t",
                                          j=2, s=2, c=CK))
                lds[pair] = t_

            def emit_kv(i):
                # stationary [Wk|Wv] (even i) or [Wv|Wk] (odd i)
                ps = kv_psum.tile([128, T], F32, tag="kv", name="kv_ps")
                wc = WKV_E if i % 2 == 0 else WKV_O
                xv = ld_view(i)
                for c in range(CK):
                    nc.tensor.matmul(
                        ps[:, :], w_sb[:, c, wc:wc + 128], xv[:, 1, c, :],
                        start=(c == 0), stop=(c == CK - 1))
                return ps

            def emit_kv_copy(i, ps):
                sb = kv_pool.tile([128, T], BF16, tag="kv_sb", name="kv_sb")
                nc.vector.tensor_copy(sb[:, 0:256], ps[:, 0:256])
                nc.scalar.copy(sb[:, 256:512], ps[:, 256:512])
                kvs[i] = sb

            def emit_q(i):
                # batches 2k/2k+1 stack into one PSUM tile at partition 0/64
                pair = i // 2
                if i % 2 == 0:
                    qps_t[pair] = q_psum.tile([128, T], F32, tag="q",
                                              name="q_ps")
                ps = qps_t[pair]
                pbase = 64 * (i % 2)
                iv = ld_view(i)
                for c in range(CK):
                    nc.tensor.matmul(
                        ps[pbase:pbase + 64, :], w_sb[:, c, WQ0:WQ0 + 64],
                        iv[:, 0, c, :],
                        start=(c == 0), stop=(c == CK - 1))

            def emit_qcopy(pair):
                sb = q_pool.tile([128, T], BF16, tag="q_sb", name="q_sb")
                nc.vector.tensor_copy(sb[:], qps_t[pair][:])
                qsbs[pair] = sb
                del qps_t[pair]

            def emit_vt(i):
                # transpose vT [64, T] -> v [t, h] chunks (PE, bf16)
                vb = 64 if i % 2 == 0 else 0
                ps = vt_psum.tile([128, TK, H], BF16, tag="vt", name="vt_ps")
                for tk in range(TK):
                    nc.tensor.transpose(
                        ps[:, tk, :], kvs[i][vb:vb + 64, 128 * tk:128 * (tk + 1)],
                        ident[vb:vb + 64, :])
                return ps

            def emit_vcopy(i, vt_ps):
                sb = v_pool.tile([128, TK, H + 1], BF16, tag="v_sb",
                                 name="v_sb")
                nc.gpsimd.memset(sb[:, :, H], 1.0)
                nc.vector.tensor_copy(sb[:, :, 0:H], vt_ps[:])
                return sb

            def emit_S(i):
                kb = 0 if i % 2 == 0 else 64
                st0 = st_psum.tile([128, T], F32, tag="st0", name="st0")
                # chunks 1,3,2 packed: m1 cols 0:384 | m3 384:512 | m2 512:768
                st123 = st_psum.tile([128, 768], F32, tag="st123",
                                     name="st123")
                kv_sb, q_sb = kvs[i], qsbs[i // 2]
                nc.tensor.matmul(st0[:, :], kv_sb[kb:kb + 64, 0:128],
                                 q_sb[kb:kb + 64, :], start=True, stop=True)
                nc.tensor.matmul(st123[:, 0:384], kv_sb[kb:kb + 64, 128:256],
                                 q_sb[kb:kb + 64, 128:T], start=True, stop=True)
                nc.tensor.matmul(st123[:, 384:512], kv_sb[kb:kb + 64, 384:512],
                                 q_sb[kb:kb + 64, 384:T], start=True, stop=True)
                nc.tensor.matmul(st123[:, 512:768], kv_sb[kb:kb + 64, 256:384],
                                 q_sb[kb:kb + 64, 256:T], start=True, stop=True)
                return st0, st123

            def emit_PV(i, e0, e123, v_sb):
                ps = o_psum.tile([128, TK, H + 1], F32, tag="o", name="o_ps")
                for t in range(TK):
                    for m in range(t + 1):
                        if m == 0:
                            lhsT = e0[:, 128 * t:128 * (t + 1)]
                        elif m == 1:
                            lhsT = e123[:, 128 * (t - 1):128 * t]
                        elif m == 2:
                            lhsT = e123[:, 512 + 128 * (t - 2):512 + 128 * (t - 1)]
                        else:
                            lhsT = e123[:, 384:512]
                        nc.tensor.matmul(ps[:, t, :], lhsT, v_sb[:, m, :],
                                         start=(m == 0), stop=(m == t))
                return ps

            # ---- prologue ----
            for p in range(3):
                emit_ld(p)
            kv_ps0 = emit_kv(0)
            emit_kv_copy(0, kv_ps0)
            emit_q(0)
            emit_q(1)
            emit_qcopy(0)

            # ---- steady-state loop ----
            for b in range(BPC):
                if b % 2 == 0:
                    emit_ld(b // 2 + 3)
                st0, st123 = emit_S(b)
                vt_ps = emit_vt(b)
                v_sb = emit_vcopy(b, vt_ps)

                e0 = e_pool.tile([128, T], BF16, tag="e0", name="e0")
                e123 = e_pool.tile([128, 768], BF16, tag="e123", name="e123")
                nc.scalar.activation(e0[:], st0[:], EXP, scale=SCALE)
                nc.vector.tensor_mul(e0[:, 0:128], e0[:, 0:128], tri[:])
                nc.scalar.activation(e123[:], st123[:], EXP, scale=SCALE)

                if b + 1 < BPC:
                    ps = emit_kv(b + 1)
                    emit_kv_copy(b + 1, ps)

                # diagonal-block masks: m1 at cols 0:128, m3|m2 at 384:640
                nc.vector.tensor_mul(e123[:, 0:128], e123[:, 0:128], tri[:])
                dia = e123[:, 384:640].rearrange("p (u v) -> p u v", u=2)
                nc.vector.tensor_mul(dia, dia, tripair[:])

                if b + 2 < BPC:
                    emit_q(b + 2)
                    if (b + 2) % 2 == 1:
                        emit_qcopy((b + 2) // 2)

                o_ps = emit_PV(b, e0, e123, v_sb)

                recip = r_pool.tile([128, TK, 1], F32, tag="recip",
                                    name="recip")
                nc.vector.reciprocal(recip[:, :, 0], o_ps[:, :, H])
                if b % 2 == 0:
                    osbs[b // 2] = o_pool.tile([128, 2, TK * H], BF16,
                                               tag="o_sb", name="o_sb")
                o_sb = osbs[b // 2]
                src, rcp = broadcast_tensor_aps(o_ps[:, :, 0:H], recip[:])
                dst = o_sb[:, b % 2].rearrange("p (t h) -> p t h", t=TK)
                nc.vector.scalar_tensor_tensor(
                    dst, src, 1.0, rcp, op0=MULT, op1=MULT)
                if b % 2 == 1:
                    nc.sync.dma_start(out[b // 2], o_sb[:, :, :])

    if split_waits:
        _split_multi_waits(nc)
    return nc


_cached_nc = None


def _pack_inputs(input, x, Wq, Wk, Wv):
    """Host-side repack: per-pair per-partition contiguous 12KiB blobs."""
    input = np.asarray(input, dtype=np.float32)
    x = np.asarray(x, dtype=np.float32)
    # [b, t, c*128+p] -> [b, p, s, c, t]
    fused = np.empty((B, 128, 2, CK, T), dtype=_bf16)
    fused[:, :, 0] = input.transpose(0, 2, 1).reshape(B, CK, 128, T).transpose(0, 2, 1, 3)
    fused[:, :, 1] = x.transpose(0, 2, 1).reshape(B, CK, 128, T).transpose(0, 2, 1, 3)
    # pair up: [B/2, 2, 128, blob] -> [B/2, 128, 2, blob]
    fused = fused.reshape(B // 2, 2, 128, 2 * CK * T).transpose(0, 2, 1, 3)
    fused = fused.reshape(B // 2, 128, 2 * 2 * CK * T)

    Wq = np.asarray(Wq, dtype=np.float32)
    Wk = np.asarray(Wk, dtype=np.float32)
    Wv = np.asarray(Wv, dtype=np.float32)
    w_all = np.concatenate(
        [np.concatenate([Wk, Wv], 1), np.concatenate([Wv, Wk], 1), Wq], axis=1)
    w_host = np.ascontiguousarray(
        w_all.reshape(CK, 128, WCOLS).transpose(1, 0, 2).reshape(128, CK * WCOLS)
    ).astype(_bf16)
    return fused, w_host


def make_in_maps(input, x, Wq, Wk, Wv):
    fused, w_host = _pack_inputs(input, x, Wq, Wk, Wv)
    in_maps = []
    for c in range(N_CORES):
        sl = slice(c * NP, (c + 1) * NP)
        in_maps.append({
            "fused": np.ascontiguousarray(fused[sl]),
            "w": w_host,
        })
    return in_maps


def _unpack_out(res_outs):
    # out [NP, 128, 2*TK*H] bf16: [pair, p, (j, t, h)] = OUT[2pair+j, 128t+p, h]
    full = np.concatenate([np.asarray(r) for r in res_outs], axis=0)
    full = full.reshape(-1, 128, 2, TK, H).transpose(0, 2, 3, 1, 4)
    return np.ascontiguousarray(full.reshape(-1, T, H).astype(np.float32))


def kernel(input: np.ndarray, x: np.ndarray, Wq: np.ndarray, Wk: np.ndarray,
           Wv: np.ndarray) -> np.ndarray:
    global _cached_nc
    if _cached_nc is None:
        _cached_nc = build_kernel()
    nc = _cached_nc

    in_maps = make_in_maps(input, x, Wq, Wk, Wv)
    res = run_bass_kernel_spmd(nc, in_maps, core_ids=list(range(N_CORES)))
    return _unpack_out([r["out"] for r in res.results])


# revision 9
# speedup vs baseline: 1.1062x; 1.1062x over previous
"""Trainium2 Bass kernel for a single-head cross-attention block.

Reference computation (per batch b of B=128):
    q = input[b] @ Wq            # [T,H]   T=512, C=384, H=64
    k = x[b] @ Wk                # [T,H]
    v = x[b] @ Wv                # [T,H]
    S = (q @ k.T) * C**-0.5      # [T,T], causal mask
    P = softmax(S, axis=-1)
    out[b] = P @ v               # [T,H]

Data-parallel over 8 NeuronCores (16 batches each).  Key structure:

  * Host pre-packs input+x for a PAIR of batches into one DRAM blob laid
    out so every SBUF partition's slice is a single 12 KiB contiguous run;
    one dma_start per pair (DMA issue costs ~600ns of engine time each,
    and small packets choke the queue dispatcher).  Outputs likewise
    store bf16 pair-blobs.  All DMA issues ride the (otherwise idle)
    sync-engine HWDGE queue.
  * k and v projections are fused into one matmul pass using a stacked
    [Wk|Wv] stationary tile: out [128,T] holds kT on partitions 0-63 and
    vT on 64-127 (halves the kv projection PE cost).  Odd batches use
    [Wv|Wk] so kT lands on partitions 64-127 - this matches the q
    projection of odd batches which is PSUM-stacked on partitions 64-127
    (two batches share one PSUM bank + one PSUM->SBUF copy).
  * v is re-oriented [t,h] with 4 tiny PE transposes (bf16, via identity).
  * S^T chunks exploit causality (chunk m covers queries >= 128m);
    exp runs on ScalarE in 2 instructions (chunk 0, chunks 1+3+2 packed
    into one PSUM tile); diagonal-block masks multiply by a 0/1 upper-
    triangular tile on DVE.
  * P@V accumulates with an extra ones-column on v so the softmax
    denominator falls out of the same matmuls; normalization is a
    broadcast multiply by the reciprocal.
  * GPSIMD cannot touch PSUM (walrus verifier), so PSUM->SBUF moves
    split between DVE and ScalarE.  Software pipelining: q projections
    run two batches ahead, kv one batch ahead, so the PE stream stays
    dense while exp/mask chase it.
"""

import numpy as np
import ml_dtypes

import concourse.bass as bass
import concourse.tile as tile
import concourse.mybir as mybir
from concourse.bass import broadcast_tensor_aps
from concourse.bass_utils import run_bass_kernel_spmd
from concourse.masks import make_upper_triangular, make_identity

N_CORES = 8
B, T, C, H = 128, 512, 384, 64
BPC = B // N_CORES          # batches per core
NP = BPC // 2               # batch pairs per core
CK = C // 128               # contraction chunks for projections
TK = T // 128               # T chunks
SCALE = float(C) ** -0.5
BF16 = mybir.dt.bfloat16
F32 = mybir.dt.float32
EXP = mybir.ActivationFunctionType.Exp
MULT = mybir.AluOpType.mult

_bf16 = ml_dtypes.bfloat16

# weight tile column layout: [Wk|Wv] (even), [Wv|Wk] (odd), Wq
WKV_E, WKV_O, WQ0 = 0, 128, 256
WCOLS = 320


def _split_multi_waits(nc: bass.Bass):
    """walrus in this build encodes at most ONE sync-wait per instruction.
    Tile's wait-assignment can attach several. Move the extras onto
    same-engine NOPs inserted immediately before each instruction —
    identical semantics (the engine blocks on the NOP waits first)."""
    n = 0
    for bb in nc.m.functions[0].blocks:
        new_insts = []
        for inst in bb.instructions:
            si = inst.sync_info
            waits = list(si.on_wait) if si and si.on_wait else []
            if len(waits) > 1:
                for w in waits[:-1]:
                    nop = mybir.InstNoOp(name=f"WSPLIT-{n}", ins=[], outs=[])
                    n += 1
                    nop.engine = inst.engine
                    nop.sync_info = mybir.SyncInfo(on_wait=[w], on_update=[])
                    new_insts.append(nop)
                si.on_wait = waits[-1:]
            new_insts.append(inst)
        bb.instructions[:] = new_insts


def build_kernel(split_waits: bool = True) -> bass.Bass:
    nc = bass.Bass()
    # fused[pair, p, (j, s, c, t)]: batch 2*pair+j, s=0 input / s=1 x,
    # value = src[b, t, c*128 + p]
    fused = nc.dram_tensor("fused", [NP, 128, 2 * 2 * CK * T], BF16,
                           kind="ExternalInput")
    w = nc.dram_tensor("w", [128, CK * WCOLS], BF16, kind="ExternalInput")
    out = nc.dram_tensor("out", [NP, 128, 2 * TK * H], BF16,
                         kind="ExternalOutput")

    with tile.TileContext(nc) as tc:
        with (
            tc.tile_pool(name="const", bufs=1) as const_pool,
            tc.tile_pool(name="ld", bufs=4) as ld_pool,
            tc.tile_pool(name="kv", bufs=2) as kv_pool,
            tc.tile_pool(name="qsb", bufs=2) as q_pool,
            tc.tile_pool(name="vsb", bufs=2) as v_pool,
            tc.tile_pool(name="e", bufs=2) as e_pool,
            tc.tile_pool(name="osb", bufs=2) as o_pool,
            tc.tile_pool(name="rcp", bufs=2) as r_pool,
            tc.tile_pool(name="q_ps", bufs=1, space="PSUM") as q_psum,
            tc.tile_pool(name="kv_ps", bufs=1, space="PSUM") as kv_psum,
            tc.tile_pool(name="vt_ps", bufs=1, space="PSUM") as vt_psum,
            tc.tile_pool(name="st_ps", bufs=1, space="PSUM") as st_psum,
            tc.tile_pool(name="o_ps", bufs=2, space="PSUM") as o_psum,
        ):
            # ---- constants ----
            w_sb = const_pool.tile([128, CK, WCOLS], BF16, tag="w")
            nc.sync.dma_start(w_sb[:], w[:].rearrange("p (c f) -> p c f", c=CK))
            tri = const_pool.tile([128, 128], BF16, tag="tri")
            make_upper_triangular(nc, tri[:], val=1.0, diag=True)
            tripair = const_pool.tile([128, 2, 128], BF16, tag="tripair")
            make_upper_triangular(nc, tripair[:, 0, :], val=1.0, diag=True)
            make_upper_triangular(nc, tripair[:, 1, :], val=1.0, diag=True)
            # identity stacked twice so both parities can transpose from
            # their base partition (0 or 64)
            ident = const_pool.tile([128, 64], BF16, tag="ident")
            make_identity(nc, ident[0:64, :])
            make_identity(nc, ident[64:128, :])

            lds, kvs, qps_t, qsbs, osbs = {}, {}, {}, {}, {}

            def ld_view(i):
                # [s, c, t] view (128 partitions) of batch i in its pair tile
                return lds[i // 2][:, i % 2]

            def emit_ld(pair):
                if pair >= NP or pair in lds:
                    return
                t_ = ld_pool.tile([128, 2, 2, CK, T], BF16, tag="ld", name="ld")
                nc.sync.dma_start(
                    t_[:],
                    fused[pair].rearrange("p (j s c t) -> p j s c t",
                                          j=2, s=2, c=CK))
                lds[pair] = t_

            def emit_kv(i):
                # stationary [Wk|Wv] (even i) or [Wv|Wk] (odd i)
                ps = kv_psum.tile([128, T], F32, tag="kv", name="kv_ps")
                wc = WKV_E if i % 2 == 0 else WKV_O
                xv = ld_view(i)
                for c in range(CK):
                    nc.tensor.matmul(
                        ps[:, :], w_sb[:, c, wc:wc + 128], xv[:, 1, c, :],
                        start=(c == 0), stop=(c == CK - 1))
                return ps

            def emit_kv_copy(i, ps):
                sb = kv_pool.tile([128, T], BF16, tag="kv_sb", name="kv_sb")
                nc.vector.tensor_copy(sb[:, 0:256], ps[:, 0:256])
                nc.scalar.copy(sb[:, 256:512], ps[:, 256:512])
                kvs[i] = sb

            def emit_q(i):
                # batches 2k/2k+1 stack into one PSUM tile at partition 0/64
                pair = i // 2
                if i % 2 == 0:
                    qps_t[pair] = q_psum.tile([128, T], F32, tag="q",
                                              name="q_ps")
                ps = qps_t[pair]
                pbase = 64 * (i % 2)
                iv = ld_view(i)
                for c in range(CK):
                    nc.tensor.matmul(
                        ps[pbase:pbase + 64, :], w_sb[:, c, WQ0:WQ0 + 64],
                        iv[:, 0, c, :],
                        start=(c == 0), stop=(c == CK - 1))

            def emit_qcopy(pair):
                sb = q_pool.tile([128, T], BF16, tag="q_sb", name="q_sb")
                nc.vector.tensor_copy(sb[:], qps_t[pair][:])
                qsbs[pair] = sb
                del qps_t[pair]

            def emit_vt(i):
                # transpose vT [64, T] -> v [t, h] chunks (PE, bf16)
                vb = 64 if i % 2 == 0 else 0
                ps = vt_psum.tile([128, TK, H], BF16, tag="vt", name="vt_ps")
                for tk in range(TK):
                    nc.tensor.transpose(
                        ps[:, tk, :], kvs[i][vb:vb + 64, 128 * tk:128 * (tk + 1)],
                        ident[vb:vb + 64, :])
                return ps

            def emit_vcopy(i, vt_ps):
                sb = v_pool.tile([128, TK, H + 1], BF16, tag="v_sb",
                                 name="v_sb")
                nc.gpsimd.memset(sb[:, :, H], 1.0)
                nc.vector.tensor_copy(sb[:, :, 0:H], vt_ps[:])
                return sb

            def emit_S(i):
                kb = 0 if i % 2 == 0 else 64
                st0 = st_psum.tile([128, T], F32, tag="st0", name="st0")
                # chunks 1,3,2 packed: m1 cols 0:384 | m3 384:512 | m2 512:768
                st123 = st_psum.tile([128, 768], F32, tag="st123",
                                     name="st123")
                kv_sb, q_sb = kvs[i], qsbs[i // 2]
                nc.tensor.matmul(st0[:, :], kv_sb[kb:kb + 64, 0:128],
                                 q_sb[kb:kb + 64, :], start=True, stop=True)
                nc.tensor.matmul(st123[:, 0:384], kv_sb[kb:kb + 64, 128:256],
                                 q_sb[kb:kb + 64, 128:T], start=True, stop=True)
                nc.tensor.matmul(st123[:, 384:512], kv_sb[kb:kb + 64, 384:512],
                                 q_sb[kb:kb + 64, 384:T], start=True, stop=True)
                nc.tensor.matmul(st123[:, 512:768], kv_sb[kb:kb + 64, 256:384],
                                 q_sb[kb:kb + 64, 256:T], start=True, stop=True)
                return st0, st123

            def emit_PV(i, e0, e123, v_sb):
                ps = o_psum.tile([128, TK, H + 1], F32, tag="o", name="o_ps")
                for t in range(TK):
                    for m in range(t + 1):
                        if m == 0:
                            lhsT = e0[:, 128 * t:128 * (t + 1)]
                        elif m == 1:
                            lhsT = e123[:, 128 * (t - 1):128 * t]
                        elif m == 2:
                            lhsT = e123[:, 512 + 128 * (t - 2):512 + 128 * (t - 1)]
                        else:
                            lhsT = e123[:, 384:512]
                        nc.tensor.matmul(ps[:, t, :], lhsT, v_sb[:, m, :],
                                         start=(m == 0), stop=(m == t))
                return ps

            # ---- prologue ----
            for p in range(3):
                emit_ld(p)
            kv_ps0 = emit_kv(0)
            emit_kv_copy(0, kv_ps0)
            emit_q(0)
            emit_q(1)
            emit_qcopy(0)

            # ---- steady-state loop: PV runs one slot behind S/exp/mask so
            # every PE instruction's deps are satisfied a full slot early ----
            work = {}   # bn -> (e0, e123, v_sb)
            for slot in range(-1, BPC):
                # store pair (data finished at end of slot 2p+1); issue on
                # scalar queue early in slot so it never delays the exps
                if slot >= 2 and slot % 2 == 0:
                    nc.scalar.dma_start(out[slot // 2 - 1], osbs[slot // 2 - 1][:, :, :])
                if slot % 2 == 0:
                    emit_ld(slot // 2 + 3)

                bn = slot + 1
                if bn < BPC:
                    st0, st123 = emit_S(bn)
                    e0 = e_pool.tile([128, T], BF16, tag="e0", name="e0")
                    e123 = e_pool.tile([128, 768], BF16, tag="e123",
                                       name="e123")
                    nc.scalar.activation(e0[:], st0[:], EXP, scale=SCALE)
                    nc.vector.tensor_mul(e0[:, 0:128], e0[:, 0:128], tri[:])
                    nc.scalar.activation(e123[:], st123[:], EXP, scale=SCALE)

                    vt_ps = emit_vt(bn)
                    v_sb = emit_vcopy(bn, vt_ps)

                    if bn + 1 < BPC:
                        ps = emit_kv(bn + 1)
                        emit_kv_copy(bn + 1, ps)

                    # diagonal-block masks: m1 at cols 0:128, m3|m2 at 384:640
                    nc.vector.tensor_mul(e123[:, 0:128], e123[:, 0:128], tri[:])
                    dia = e123[:, 384:640].rearrange("p (u v) -> p u v", u=2)
                    nc.vector.tensor_mul(dia, dia, tripair[:])

                    if bn + 2 < BPC:
                        emit_q(bn + 2)
                        if (bn + 2) % 2 == 1:
                            emit_qcopy((bn + 2) // 2)
                    work[bn] = (e0, e123, v_sb)

                if slot >= 0:
                    e0, e123, v_sb = work.pop(slot)
                    o_ps = emit_PV(slot, e0, e123, v_sb)
                    recip = r_pool.tile([128, TK, 1], F32, tag="recip",
                                        name="recip")
                    nc.vector.reciprocal(recip[:, :, 0], o_ps[:, :, H])
                    if slot % 2 == 0:
                        osbs[slot // 2] = o_pool.tile([128, 2, TK * H], BF16,
                                                      tag="o_sb", name="o_sb")
                    o_sb = osbs[slot // 2]
                    src, rcp = broadcast_tensor_aps(o_ps[:, :, 0:H], recip[:])
                    dst = o_sb[:, slot % 2].rearrange("p (t h) -> p t h", t=TK)
                    nc.vector.scalar_tensor_tensor(
                        dst, src, 1.0, rcp, op0=MULT, op1=MULT)
            nc.scalar.dma_start(out[NP - 1], osbs[NP - 1][:, :, :])

    if split_waits:
        _split_multi_waits(nc)
    return nc


_cached_nc = None


def _pack_inputs(input, x, Wq, Wk, Wv):
    """Host-side repack: per-pair per-partition contiguous 12KiB blobs."""
    input = np.asarray(input, dtype=np.float32)
    x = np.asarray(x, dtype=np.float32)
    # [b, t, c*128+p] -> [b, p, s, c, t]
    fused = np.empty((B, 128, 2, CK, T), dtype=_bf16)
    fused[:, :, 0] = input.transpose(0, 2, 1).reshape(B, CK, 128, T).transpose(0, 2, 1, 3)
    fused[:, :, 1] = x.transpose(0, 2, 1).reshape(B, CK, 128, T).transpose(0, 2, 1, 3)
    # pair up: [B/2, 2, 128, blob] -> [B/2, 128, 2, blob]
    fused = fused.reshape(B // 2, 2, 128, 2 * CK * T).transpose(0, 2, 1, 3)
    fused = fused.reshape(B // 2, 128, 2 * 2 * CK * T)

    Wq = np.asarray(Wq, dtype=np.float32)
    Wk = np.asarray(Wk, dtype=np.float32)
    Wv = np.asarray(Wv, dtype=np.float32)
    w_all = np.concatenate(
        [np.concatenate([Wk, Wv], 1), np.concatenate([Wv, Wk], 1), Wq], axis=1)
    w_host = np.ascontiguousarray(
        w_all.reshape(CK, 128, WCOLS).transpose(1, 0, 2).reshape(128, CK * WCOLS)
    ).astype(_bf16)
    return fused, w_host


def make_in_maps(input, x, Wq, Wk, Wv):
    fused, w_host = _pack_inputs(input, x, Wq, Wk, Wv)
    in_maps = []
    for c in range(N_CORES):
        sl = slice(c * NP, (c + 1) * NP)
        in_maps.append({
            "fused": np.ascontiguousarray(fused[sl]),
            "w": w_host,
        })
    return in_maps


def _unpack_out(res_outs):
    # out [NP, 128, 2*TK*H] bf16: [pair, p, (j, t, h)] = OUT[2pair+j, 128t+p, h]
    full = np.concatenate([np.asarray(r) for r in res_outs], axis=0)
    full = full.reshape(-1, 128, 2, TK, H).transpose(0, 2, 3, 1, 4)
    return np.ascontiguousarray(full.reshape(-1, T, H).astype(np.float32))


def kernel(input: np.ndarray, x: np.ndarray, Wq: np.ndarray, Wk: np.ndarray,
           Wv: np.ndarray) -> np.ndarray:
    global _cached_nc
    if _cached_nc is None:
        _cached_nc = build_kernel()
    nc = _cached_nc

    in_maps = make_in_maps(input, x, Wq, Wk, Wv)
    res = run_bass_kernel_spmd(nc, in_maps, core_ids=list(range(N_CORES)))
    return _unpack_out([r["out"] for r in res.results])
